# revision 6
# baseline (speedup 1.0000x reference)
"""Fused Conv3x3 + BatchNorm(train) + ReLU on 8 TRN2 NeuronCores.

Data-parallel over batch: each core processes 8 of the 64 images.
Conv is computed as matmuls over PSUM tiles of [128 out_ch, 512 pixels]:
the 9 filter taps are covered by 3 K=128 matmuls (kh=0,1 paired on the
partition axis) plus 3 K=64 matmuls (kh=2), accumulating in PSUM.
BatchNorm batch statistics (sum, sum-of-squares per channel) are reduced
across cores with a single small AllReduce, then scale/shift + ReLU are
applied in one scalar-engine activation pass per output chunk.
"""

import numpy as np

import concourse.bacc as bacc
import concourse.tile as tile
from concourse import mybir
from concourse.bass_utils import run_bass_kernel_spmd

N_CORES = 8
IMG_PER_CORE = 8          # 64 images / 8 cores
C_IN = 64
C_OUT = 128
H = W = 64
HP, WP = H + 2, W + 2     # zero-padded image
PIX = H * W               # 4096
TILE_PX = 512             # one PSUM bank of fp32
ROWS_PER_TILE = TILE_PX // W       # 8
TILES_PER_IMG = PIX // TILE_PX     # 8
N_TILES = IMG_PER_CORE * TILES_PER_IMG  # 64
BN_EPS = 1e-5
COUNT = 64 * H * W        # batch-stat count over (N, H, W)

F32 = mybir.dt.float32
F32R = mybir.dt.float32r

# Set by test harness to capture a profile; LAST_EXEC_NS holds the result.
KERNEL_TRACE = False
LAST_EXEC_NS = None

_cached_nc = None


def _build():
    nc = bacc.Bacc("TRN2", target_bir_lowering=False, debug=False,
                   num_devices=N_CORES)

    x_in = nc.dram_tensor("x", [IMG_PER_CORE, C_IN, HP * WP], F32R,
                          kind="ExternalInput")
    wt_in = nc.dram_tensor("wt", [128, 6, 128], F32R, kind="ExternalInput")
    gb_in = nc.dram_tensor("gb", [128, 2], F32, kind="ExternalInput")
    out_d = nc.dram_tensor("out", [IMG_PER_CORE, C_OUT, PIX], F32,
                           kind="ExternalOutput")
    cc_in = nc.dram_tensor("cc_in", [128, 2], F32)
    cc_out = nc.dram_tensor("cc_out", [128, 2], F32, addr_space="Shared")

    with tile.TileContext(nc) as tc:
        with (
            tc.tile_pool(name="consts", bufs=1) as consts,
            tc.tile_pool(name="xx", bufs=2) as xx_pool,
            tc.tile_pool(name="ybuf", bufs=1) as ybuf_pool,
            tc.tile_pool(name="scratch", bufs=2) as scratch_pool,
            tc.tile_pool(name="stats", bufs=1) as stats_pool,
            tc.tile_pool(name="outp", bufs=2) as out_pool,
            tc.tile_pool(name="psum", bufs=4, space="PSUM") as psum_pool,
        ):
            wt = consts.tile([128, 6, 128], F32R)
            nc.sync.dma_start(out=wt[:], in_=wt_in[:])
            gb = consts.tile([128, 2], F32)
            nc.sync.dma_start(out=gb[:], in_=gb_in[:])
            eps_t = consts.tile([128, 1], F32)
            nc.vector.memset(eps_t[:], BN_EPS)

            # y stays resident in SBUF between the conv and the BN apply.
            ybuf = ybuf_pool.tile([128, N_TILES, TILE_PX], F32)
            sums = stats_pool.tile([128, N_TILES], F32)
            sumsqs = stats_pool.tile([128, N_TILES], F32)

            for img in range(IMG_PER_CORE):
                # xx: padded image, channels on partitions 0-63; partitions
                # 64-127 hold the same image shifted down one padded row so
                # (kh=0, kh=1) taps pair into one K=128 contraction.
                xx = xx_pool.tile([128, HP, WP], F32R)
                # host delivers the image pre-padded: one contiguous DMA
                nc.sync.dma_start(
                    out=xx[0:64, :, :].rearrange("p a b -> p (a b)"),
                    in_=x_in[img])
                # shifted copy: upper[r] = lower[r+1], rows 0..HP-2
                nc.sync.dma_start(out=xx[64:128, 0:HP - 1, :],
                                  in_=xx[0:64, 1:HP, :])

                for t in range(TILES_PER_IMG):
                    h0 = t * ROWS_PER_TILE
                    gt = img * TILES_PER_IMG + t
                    ps = psum_pool.tile([128, TILE_PX], F32)
                    # kh=2 singles first: K=64 -> K=128 within one PSUM
                    # accumulation group is safe; the reverse order
                    # (shrinking K mid-group) hangs the exec unit.
                    for kw in range(3):
                        nc.tensor.matmul(
                            ps[:],
                            lhsT=wt[0:64, 3 + kw, :],
                            rhs=xx[0:64, h0 + 2:h0 + 2 + ROWS_PER_TILE,
                                   kw:kw + W],
                            start=(kw == 0),
                            stop=False,
                        )
                    # kh=0,1 pairs: K=128
                    for kw in range(3):
                        nc.tensor.matmul(
                            ps[:],
                            lhsT=wt[:, kw, :],
                            rhs=xx[:, h0:h0 + ROWS_PER_TILE,
                                   kw:kw + W],
                            start=False,
                            stop=(kw == 2),
                        )
                    # PSUM -> SBUF copy + per-channel sum
                    nc.scalar.activation(
                        ybuf[:, gt, :], ps[:],
                        mybir.ActivationFunctionType.Copy,
                        accum_out=sums[:, gt:gt + 1],
                    )
                    # square + per-channel sum of squares (on DVE)
                    sc = scratch_pool.tile([128, TILE_PX], F32)
                    nc.vector.tensor_mul(sc[:], ybuf[:, gt, :],
                                         ybuf[:, gt, :])
                    nc.vector.reduce_sum(sumsqs[:, gt:gt + 1], sc[:],
                                         axis=mybir.AxisListType.X)

            # fold per-tile partials, all-reduce across the 8 cores
            st = stats_pool.tile([128, 2], F32)
            nc.vector.reduce_sum(st[:, 0:1], sums[:],
                                 axis=mybir.AxisListType.X)
            nc.vector.reduce_sum(st[:, 1:2], sumsqs[:],
                                 axis=mybir.AxisListType.X)
            nc.sync.dma_start(out=cc_in[:], in_=st[:])
            nc.gpsimd.collective_compute(
                "AllReduce",
                mybir.AluOpType.add,
                ins=[cc_in[:]],
                outs=[cc_out[:]],
                replica_groups=[list(range(N_CORES))],
            )
            g = stats_pool.tile([128, 2], F32)
            nc.sync.dma_start(out=g[:], in_=cc_out[:])

            # scale = gamma * rsqrt(var + eps); shift = beta - scale * mean
            mean = stats_pool.tile([128, 1], F32)
            m2 = stats_pool.tile([128, 1], F32)
            var = stats_pool.tile([128, 1], F32)
            sd = stats_pool.tile([128, 1], F32)
            inv = stats_pool.tile([128, 1], F32)
            scl = stats_pool.tile([128, 1], F32)
            shv = stats_pool.tile([128, 1], F32)
            tmp = stats_pool.tile([128, 1], F32)
            nc.vector.tensor_scalar_mul(mean[:], g[:, 0:1], 1.0 / COUNT)
            nc.vector.tensor_scalar_mul(m2[:], g[:, 1:2], 1.0 / COUNT)
            nc.vector.tensor_mul(tmp[:], mean[:], mean[:])
            nc.vector.tensor_sub(var[:], m2[:], tmp[:])
            nc.scalar.activation(sd[:], var[:],
                                 mybir.ActivationFunctionType.Sqrt,
                                 bias=eps_t[:])
            nc.vector.reciprocal(inv[:], sd[:])
            nc.vector.tensor_mul(scl[:], gb[:, 0:1], inv[:])
            nc.vector.tensor_mul(tmp[:], scl[:], mean[:])
            nc.vector.tensor_sub(shv[:], gb[:, 1:2], tmp[:])

            # apply: out = relu(y * scale + shift), in half-image chunks
            CH_TILES = 4  # tiles per chunk
            for img in range(IMG_PER_CORE):
                for half in range(TILES_PER_IMG // CH_TILES):
                    t0 = img * TILES_PER_IMG + half * CH_TILES
                    ot = out_pool.tile([128, CH_TILES, TILE_PX], F32)
                    nc.scalar.activation(
                        ot[:], ybuf[:, t0:t0 + CH_TILES, :],
                        mybir.ActivationFunctionType.Relu,
                        bias=shv[:], scale=scl[:],
                    )
                    px0 = half * CH_TILES * TILE_PX
                    nc.sync.dma_start(
                        out=out_d[img, :, px0:px0 + CH_TILES * TILE_PX],
                        in_=ot[:],
                    )

    nc.compile()
    return nc


def _prep_weights(weight: np.ndarray) -> np.ndarray:
    # [p, q, mb, mb] block matrix -> truncated OIHW kernel [128, 64, 3, 3]
    p, q, mb, _ = weight.shape
    Wm = weight.transpose(0, 2, 1, 3).reshape(p * mb, q * mb)
    Wm = Wm[:C_OUT, :C_IN * 9].reshape(C_OUT, C_IN, 3, 3)
    wt = np.zeros((128, 6, 128), np.float32)
    # pairs: partition c -> (kh=0), partition 64+c -> (kh=1)
    wt[:64, 0:3, :] = Wm[:, :, 0, :].transpose(1, 2, 0)
    wt[64:, 0:3, :] = Wm[:, :, 1, :].transpose(1, 2, 0)
    # singles (kh=2), duplicated in both partition halves
    wt[:64, 3:6, :] = Wm[:, :, 2, :].transpose(1, 2, 0)
    wt[64:, 3:6, :] = Wm[:, :, 2, :].transpose(1, 2, 0)
    return wt


def kernel(x, weight, gamma, beta):
    global _cached_nc, LAST_EXEC_NS
    x = np.asarray(x, np.float32)
    weight = np.asarray(weight, np.float32)
    gamma = np.asarray(gamma, np.float32)
    beta = np.asarray(beta, np.float32)

    if _cached_nc is None:
        _cached_nc = _build()
    nc = _cached_nc

    wt = _prep_weights(weight)
    gb = np.ascontiguousarray(np.stack([gamma, beta], axis=1))
    xp = np.zeros((64, C_IN, HP, WP), np.float32)
    xp[:, :, 1:H + 1, 1:W + 1] = x
    xp = xp.reshape(64, C_IN, HP * WP)
    in_maps = []
    for i in range(N_CORES):
        shard = np.ascontiguousarray(
            xp[i * IMG_PER_CORE:(i + 1) * IMG_PER_CORE])
        in_maps.append({"x": shard, "wt": wt, "gb": gb})

    res = run_bass_kernel_spmd(nc, in_maps, list(range(N_CORES)),
                               trace=KERNEL_TRACE)
    LAST_EXEC_NS = res.exec_time_ns

    out = np.concatenate(
        [res.results[i]["out"].reshape(IMG_PER_CORE, C_OUT, H, W)
         for i in range(N_CORES)], axis=0)
    return out
